# revision 5
# baseline (speedup 1.0000x reference)
"""MiniSTU Trainium2 kernel (8 NeuronCores, Bass/Tile).

Math: the reference's FFT convolution + einsum collapses to
    y[b,l,o] = sum_g sum_{t<=l} phi_eff_g[l-t] * (x[b,t] @ M_g)[o]
over g in the 48 (filter k, sign) pairs, where phi_eff carries the
(-1)^s alternation for the minus branch (the two sgn factors in the
reference combine to (-1)^(l-t), i.e. an alternating filter).

Device algorithm per core (6 pairs per core, filter-dim sharding):
  stage 1: Z_g[t, (b,o)] = xT_tile.T @ M_g       (PE, f32r)
  stage 2: y[c] += Toeplitz(phi_eff_g)[c-cp].T @ Z_g[cp]   (PE, f32r)
Toeplitz blocks are expanded on host from phi. The 8 per-core partial
outputs are summed on host (the gather for this sharding).
"""

import numpy as np

import concourse.bass as bass
import concourse.tile as tile
from concourse import mybir
from concourse.bass_utils import run_bass_kernel_spmd
from concourse.vector_clock import ScopedClock

L = 2048
K = 24
I = 256
O = 256
B = 2
TS = 128          # tile size along sequence
CT = L // TS      # 16 sequence tiles
NP = 6            # (k, sign) pairs per core
N_CORES = 8
BO = B * O        # 512 fused (b, o) columns
F32 = mybir.dt.float32
F32R = mybir.dt.float32r


# ---------------------------------------------------------------------------
# Workarounds for this container's walrus: it rejects any instruction that
# carries more than one sync-wait command.
# ---------------------------------------------------------------------------

def _split_sync_waits(nc, max_waits=1):
    """Hoist extra sem-waits onto same-engine NOPs inserted right before the
    offending instruction; queue order keeps the semantics identical."""
    for f in nc.m.functions:
        for blk in f.blocks:
            insts = list(blk.instructions)
            out = []
            changed = False
            for inst in insts:
                si = getattr(inst, "sync_info", None)
                waits = list(si.on_wait) if si is not None else []
                if len(waits) > max_waits:
                    changed = True
                    extra, keep = waits[:-max_waits], waits[-max_waits:]
                    for j in range(0, len(extra), max_waits):
                        nop = mybir.InstNoOp(
                            name=nc.get_next_instruction_name(), ins=[], outs=[]
                        )
                        nop.engine = inst.engine
                        nop.sync_info = mybir.SyncInfo(
                            on_wait=extra[j : j + max_waits], on_update=[]
                        )
                        out.append(nop)
                    inst.sync_info = mybir.SyncInfo(
                        on_wait=keep, on_update=list(si.on_update)
                    )
                out.append(inst)
            if changed:
                blk.instructions = out


class _TC(tile.TileContext):
    """TileContext whose tail drain spreads its waits over 1-wait NOPs."""

    def _drain_and_barrier(self, tick_clock, wait_clock):
        nc = self.nc
        nop_inst = nc.sync.nop()
        wait_clock.add_sem_waits(
            nop_inst.ins, ScopedClock({None: tick_clock.global_clock})
        )
        si = nop_inst.ins.sync_info
        if si is not None and len(si.on_wait) > 1:
            waits = list(si.on_wait)
            nop_inst.ins.sync_info = mybir.SyncInfo(
                on_wait=waits[:1], on_update=list(si.on_update)
            )
            for w in waits[1:]:
                extra = nc.sync.nop().ins
                extra.sync_info = mybir.SyncInfo(on_wait=[w], on_update=[])
        nc.sync.drain()
        nc.all_engine_barrier()
        assert self.sems is not None
        popped = nc._tile_sem_poison_stack.pop()
        assert popped is self._sem_poison
        nc.clear_and_free_semaphores(list(self.sems.allocated().values()))
        nc.all_engine_barrier()


# ---------------------------------------------------------------------------
# Device program (identical on all 8 cores; per-core data differs)
# ---------------------------------------------------------------------------

def _build_nc():
    nc = bass.Bass("TRN2", target_bir_lowering=False, debug=False,
                   num_devices=N_CORES)
    xT_d = nc.dram_tensor("xT", [B, 2, TS, L], F32R, kind="ExternalInput")
    m_d = nc.dram_tensor("m", [NP, 2, TS, O], F32R, kind="ExternalInput")
    tb_d = nc.dram_tensor("tb", [NP, CT, TS, TS], F32R, kind="ExternalInput")
    yp_d = nc.dram_tensor("yp", [CT, TS, BO], F32, kind="ExternalOutput")

    with _TC(nc) as tc:
        with (
            tc.tile_pool(name="const", bufs=1) as cpool,
            tc.tile_pool(name="ys", bufs=1) as ypool,
            tc.tile_pool(name="z", bufs=12) as zpool,
            tc.tile_pool(name="ps1", bufs=3, space="PSUM") as ps1,
            tc.tile_pool(name="ps2", bufs=4, space="PSUM") as ps2,
        ):
            xs = [[cpool.tile([TS, L], F32R, tag=f"x{b}{ic}", name=f"x{b}{ic}") for ic in range(2)]
                  for b in range(B)]
            for b in range(B):
                for ic in range(2):
                    nc.gpsimd.dma_start(xs[b][ic][:], xT_d[b, ic])
            ms = [[cpool.tile([TS, O], F32R, tag=f"m{p}{ic}", name=f"m{p}{ic}") for ic in range(2)]
                  for p in range(NP)]
            for p in range(NP):
                for ic in range(2):
                    nc.gpsimd.dma_start(ms[p][ic][:], m_d[p, ic])
            tbs = [[cpool.tile([TS, TS], F32R, tag=f"t{p}{d}", name=f"t{p}{d}") for d in range(CT)]
                   for p in range(NP)]
            for d in range(CT):          # d-major: earliest-needed first
                for p in range(NP):
                    nc.gpsimd.dma_start(tbs[p][d][:], tb_d[p, d])

            y_sb = [ypool.tile([TS, BO], F32, tag=f"y{c}", name=f"ysb{c}") for c in range(CT)]

            for cp in range(CT):
                # stage 1: Z_p = x_tile @ M_p for this sequence tile
                zts = []
                for p in range(NP):
                    ps = ps1.tile([TS, BO], F32, tag="s1")
                    for b in range(B):
                        for ic in range(2):
                            nc.tensor.matmul(
                                ps[:, b * O:(b + 1) * O],
                                xs[b][ic][:, cp * TS:(cp + 1) * TS],
                                ms[p][ic][:],
                                start=(ic == 0),
                                stop=(ic == 1),
                            )
                    z = zpool.tile([TS, BO], F32R, tag="z")
                    nc.vector.tensor_copy(z[:], ps[:])
                    zts.append(z)
                # stage 2: scatter this tile's contribution to all c >= cp
                for c in range(cp, CT):
                    yps = ps2.tile([TS, BO], F32, tag="s2")
                    for p in range(NP):
                        nc.tensor.matmul(
                            yps[:],
                            tbs[p][c - cp][:],
                            zts[p][:],
                            start=(p == 0),
                            stop=(p == NP - 1),
                        )
                    if cp == 0:
                        nc.vector.tensor_copy(y_sb[c][:], yps[:])
                    else:
                        nc.vector.tensor_add(y_sb[c][:], y_sb[c][:], yps[:])
                    if c == cp:  # y_sb[cp] just received its last contribution
                        nc.gpsimd.dma_start(yp_d[cp], y_sb[cp][:])

    _split_sync_waits(nc)
    return nc


# ---------------------------------------------------------------------------
# Host side: input staging, sharding, gather
# ---------------------------------------------------------------------------

def _build_toeplitz(phi_eff):
    """tb[d, t, l] = phi_eff[d*TS + l - t] (0 where the index is negative)."""
    pad = np.zeros(L + TS - 1, np.float32)
    pad[TS - 1:] = phi_eff
    d = np.arange(CT)[:, None, None]
    t = np.arange(TS)[None, :, None]
    l = np.arange(TS)[None, None, :]
    return pad[d * TS + l - t + TS - 1]


_last_in_maps = None  # stashed for external profiling harnesses


def kernel(x, phi, M_phi_plus, M_phi_minus):
    global _last_in_maps
    x = np.asarray(x, np.float32)
    phi = np.asarray(phi, np.float32)
    Mp = np.asarray(M_phi_plus, np.float32)
    Mm = np.asarray(M_phi_minus, np.float32)

    xT = np.ascontiguousarray(x.transpose(0, 2, 1)).reshape(B, 2, TS, L)
    sgn = ((-1.0) ** np.arange(L)).astype(np.float32)

    m_all = np.empty((2 * K, 2, TS, O), np.float32)
    tb_all = np.empty((2 * K, CT, TS, TS), np.float32)
    for g in range(2 * K):
        k, s = g // 2, g % 2
        m_all[g] = (Mm if s else Mp)[k].reshape(2, TS, O)
        phi_eff = phi[:, k] * (sgn if s else 1.0)
        tb_all[g] = _build_toeplitz(phi_eff)

    nc = _build_nc()
    in_maps = []
    for core in range(N_CORES):
        gs = slice(core * NP, (core + 1) * NP)
        in_maps.append({
            "xT": xT,
            "m": np.ascontiguousarray(m_all[gs]),
            "tb": np.ascontiguousarray(tb_all[gs]),
        })
    _last_in_maps = in_maps
    res = run_bass_kernel_spmd(nc, in_maps, list(range(N_CORES)))
    y = np.zeros((CT, TS, B, O), np.float64)
    for core in range(N_CORES):
        y += res.results[core]["yp"].reshape(CT, TS, B, O)
    return np.ascontiguousarray(
        y.transpose(2, 0, 1, 3).reshape(B, L, O)
    ).astype(np.float32)


# revision 6
# speedup vs baseline: 1.0852x; 1.0852x over previous
"""MiniSTU Trainium2 kernel (8 NeuronCores, Bass/Tile).

Math: the reference's FFT convolution + einsum collapses to
    y[b,l,o] = sum_g sum_{t<=l} phi_eff_g[l-t] * (x[b,t] @ M_g)[o]
over g in the 48 (filter k, sign) pairs, where phi_eff carries the
(-1)^s alternation for the minus branch (the two sgn factors in the
reference combine to (-1)^(l-t), i.e. an alternating filter).

Device algorithm per core (6 pairs per core, filter-dim sharding):
  stage 1: Z_g[t, (b,o)] = xT_tile.T @ M_g       (PE, f32r)
  stage 2: y[c] += Toeplitz(phi_eff_g)[c-cp].T @ Z_g[cp]   (PE, f32r)
Toeplitz blocks are expanded on host from phi. The 8 per-core partial
outputs are summed on host (the gather for this sharding).
"""

import numpy as np

import concourse.bass as bass
import concourse.tile as tile
from concourse import mybir
from concourse.bass_utils import run_bass_kernel_spmd
from concourse.vector_clock import ScopedClock

L = 2048
K = 24
I = 256
O = 256
B = 2
TS = 128          # tile size along sequence
CT = L // TS      # 16 sequence tiles
NP = 6            # (k, sign) pairs per core
N_CORES = 8
BO = B * O        # 512 fused (b, o) columns
F32 = mybir.dt.float32
F32R = mybir.dt.float32r


# ---------------------------------------------------------------------------
# Workarounds for this container's walrus: it rejects any instruction that
# carries more than one sync-wait command.
# ---------------------------------------------------------------------------

def _split_sync_waits(nc, max_waits=1):
    """Hoist extra sem-waits onto same-engine NOPs inserted right before the
    offending instruction; queue order keeps the semantics identical."""
    for f in nc.m.functions:
        for blk in f.blocks:
            insts = list(blk.instructions)
            out = []
            changed = False
            for inst in insts:
                si = getattr(inst, "sync_info", None)
                waits = list(si.on_wait) if si is not None else []
                if len(waits) > max_waits:
                    changed = True
                    extra, keep = waits[:-max_waits], waits[-max_waits:]
                    for j in range(0, len(extra), max_waits):
                        nop = mybir.InstNoOp(
                            name=nc.get_next_instruction_name(), ins=[], outs=[]
                        )
                        nop.engine = inst.engine
                        nop.sync_info = mybir.SyncInfo(
                            on_wait=extra[j : j + max_waits], on_update=[]
                        )
                        out.append(nop)
                    inst.sync_info = mybir.SyncInfo(
                        on_wait=keep, on_update=list(si.on_update)
                    )
                out.append(inst)
            if changed:
                blk.instructions = out


class _TC(tile.TileContext):
    """TileContext whose tail drain spreads its waits over 1-wait NOPs."""

    def _drain_and_barrier(self, tick_clock, wait_clock):
        nc = self.nc
        nop_inst = nc.sync.nop()
        wait_clock.add_sem_waits(
            nop_inst.ins, ScopedClock({None: tick_clock.global_clock})
        )
        si = nop_inst.ins.sync_info
        if si is not None and len(si.on_wait) > 1:
            waits = list(si.on_wait)
            nop_inst.ins.sync_info = mybir.SyncInfo(
                on_wait=waits[:1], on_update=list(si.on_update)
            )
            for w in waits[1:]:
                extra = nc.sync.nop().ins
                extra.sync_info = mybir.SyncInfo(on_wait=[w], on_update=[])
        nc.sync.drain()
        nc.all_engine_barrier()
        assert self.sems is not None
        popped = nc._tile_sem_poison_stack.pop()
        assert popped is self._sem_poison
        nc.clear_and_free_semaphores(list(self.sems.allocated().values()))
        nc.all_engine_barrier()


# ---------------------------------------------------------------------------
# Device program (identical on all 8 cores; per-core data differs)
# ---------------------------------------------------------------------------

def _build_nc():
    nc = bass.Bass("TRN2", target_bir_lowering=False, debug=False,
                   num_devices=N_CORES)
    xT_d = nc.dram_tensor("xT", [B, 2, TS, L], F32R, kind="ExternalInput")
    m_d = nc.dram_tensor("m", [NP, 2, TS, O], F32R, kind="ExternalInput")
    tb_d = nc.dram_tensor("tb", [NP, CT, TS, TS], F32R, kind="ExternalInput")
    yp_d = nc.dram_tensor("yp", [CT, TS, BO], F32, kind="ExternalOutput")

    with _TC(nc) as tc:
        with (
            tc.tile_pool(name="const", bufs=1) as cpool,
            tc.tile_pool(name="ys", bufs=1) as ypool,
            tc.tile_pool(name="z", bufs=12) as zpool,
            tc.tile_pool(name="ps1", bufs=3, space="PSUM") as ps1,
            tc.tile_pool(name="ps2", bufs=4, space="PSUM") as ps2,
        ):
            xs = [[cpool.tile([TS, L], F32R, tag=f"x{b}{ic}", name=f"x{b}{ic}") for ic in range(2)]
                  for b in range(B)]
            for b in range(B):
                for ic in range(2):
                    nc.sync.dma_start(xs[b][ic][:], xT_d[b, ic])
            ms = [[cpool.tile([TS, O], F32R, tag=f"m{p}{ic}", name=f"m{p}{ic}") for ic in range(2)]
                  for p in range(NP)]
            for p in range(NP):
                for ic in range(2):
                    nc.gpsimd.dma_start(ms[p][ic][:], m_d[p, ic])  # SW queue
            tbs = [[cpool.tile([TS, TS], F32R, tag=f"t{p}{d}", name=f"t{p}{d}") for d in range(CT)]
                   for p in range(NP)]
            for d in range(CT):          # d-major: earliest-needed first
                for p in range(NP):
                    eng = nc.sync if (d * NP + p) % 2 == 0 else nc.gpsimd
                    eng.dma_start(tbs[p][d][:], tb_d[p, d])

            y_sb = [ypool.tile([TS, BO], F32, tag=f"y{c}", name=f"ysb{c}") for c in range(CT)]

            for cp in range(CT):
                # stage 1: Z_p = x_tile @ M_p for this sequence tile
                zts = []
                for p in range(NP):
                    ps = ps1.tile([TS, BO], F32, tag="s1")
                    for b in range(B):
                        for ic in range(2):
                            nc.tensor.matmul(
                                ps[:, b * O:(b + 1) * O],
                                xs[b][ic][:, cp * TS:(cp + 1) * TS],
                                ms[p][ic][:],
                                start=(ic == 0),
                                stop=(ic == 1),
                            )
                    z = zpool.tile([TS, BO], F32R, tag="z")
                    nc.vector.tensor_copy(z[:], ps[:])
                    zts.append(z)
                # stage 2: scatter this tile's contribution to all c >= cp
                for c in range(cp, CT):
                    yps = ps2.tile([TS, BO], F32, tag="s2")
                    for p in range(NP):
                        nc.tensor.matmul(
                            yps[:],
                            tbs[p][c - cp][:],
                            zts[p][:],
                            start=(p == 0),
                            stop=(p == NP - 1),
                        )
                    if cp == 0:
                        nc.vector.tensor_copy(y_sb[c][:], yps[:])
                    else:
                        nc.vector.tensor_add(y_sb[c][:], y_sb[c][:], yps[:])
                    if c == cp:  # y_sb[cp] just received its last contribution
                        nc.sync.dma_start(yp_d[cp], y_sb[cp][:])

    _split_sync_waits(nc)
    return nc


# ---------------------------------------------------------------------------
# Host side: input staging, sharding, gather
# ---------------------------------------------------------------------------

def _build_toeplitz(phi_eff):
    """tb[d, t, l] = phi_eff[d*TS + l - t] (0 where the index is negative)."""
    pad = np.zeros(L + TS - 1, np.float32)
    pad[TS - 1:] = phi_eff
    d = np.arange(CT)[:, None, None]
    t = np.arange(TS)[None, :, None]
    l = np.arange(TS)[None, None, :]
    return pad[d * TS + l - t + TS - 1]


_last_in_maps = None  # stashed for external profiling harnesses


def kernel(x, phi, M_phi_plus, M_phi_minus):
    global _last_in_maps
    x = np.asarray(x, np.float32)
    phi = np.asarray(phi, np.float32)
    Mp = np.asarray(M_phi_plus, np.float32)
    Mm = np.asarray(M_phi_minus, np.float32)

    xT = np.ascontiguousarray(x.transpose(0, 2, 1)).reshape(B, 2, TS, L)
    sgn = ((-1.0) ** np.arange(L)).astype(np.float32)

    m_all = np.empty((2 * K, 2, TS, O), np.float32)
    tb_all = np.empty((2 * K, CT, TS, TS), np.float32)
    for g in range(2 * K):
        k, s = g // 2, g % 2
        m_all[g] = (Mm if s else Mp)[k].reshape(2, TS, O)
        phi_eff = phi[:, k] * (sgn if s else 1.0)
        tb_all[g] = _build_toeplitz(phi_eff)

    nc = _build_nc()
    in_maps = []
    for core in range(N_CORES):
        gs = slice(core * NP, (core + 1) * NP)
        in_maps.append({
            "xT": xT,
            "m": np.ascontiguousarray(m_all[gs]),
            "tb": np.ascontiguousarray(tb_all[gs]),
        })
    _last_in_maps = in_maps
    res = run_bass_kernel_spmd(nc, in_maps, list(range(N_CORES)))
    y = np.zeros((CT, TS, B, O), np.float64)
    for core in range(N_CORES):
        y += res.results[core]["yp"].reshape(CT, TS, B, O)
    return np.ascontiguousarray(
        y.transpose(2, 0, 1, 3).reshape(B, L, O)
    ).astype(np.float32)
